# revision 1
# baseline (speedup 1.0000x reference)
"""NormalizeAggregator TRN2 Bass kernel (8-core SPMD, node-sharded).

kernel(**inputs) -> np.ndarray [100000, 128] float32

Math per node n (K=16 mailbox slots, D=128, E=8 edge types):
  count0[n,:]  = e_count[n,0,:]
  T[n]         = sum_e count0[n,e]
  r[n,k]       = 1 / count0[n, e_type[n,k]]
  a[n,:]       = sum_k r[n,k] * msg[n,k,:]
  s[n,:]       = (sum_k msg[n,k,:]) / T[n]
  out[n,:]     = [a @ W1 + b1 , s @ W2 + b2]

Device strategy (per core): supertile = 128 nodes = 16 groups x 8 nodes.
Per group the msg block [(8n x 16k) = 128, 128d] is the matmul stationary
operand; rhs is a [128, 16] block-diagonal weight matrix (8 cols of r,
8 cols of q = 1/T) so psum holds a^T / (s/T)^T directly in the orientation
stage 2 needs. Stage 2 accumulates rank-1 bias + aT.T @ W1 | qsT.T @ W2
into one [128 nodes, 128] psum tile. The r/q weights are built on-device
via one-hot compare (types vs iota), mask*count, segmented reduce and
reciprocal, operating in the (node-in-group, k) partition layout that the
host-side layout prep delivers.
"""

import sys
from contextlib import ExitStack

import numpy as np

if "/opt/trn_rl_repo" not in sys.path:
    sys.path.insert(0, "/opt/trn_rl_repo")

N = 100000
K = 16
D = 128
E = 8
N_CORES = 8
NODES_PER_ST = 128
GROUPS = 16
GNODES = 8
N_ST = 98  # supertiles per core
NPC = N_ST * NODES_PER_ST  # 12544 nodes per core
N_PAD = NPC * N_CORES  # 100352

_PROG = None


def _build_program():
    from concourse import bacc, bass, mybir, tile

    F32 = mybir.dt.float32
    nc = bacc.Bacc("TRN2", target_bir_lowering=False, debug=False, num_devices=N_CORES)

    msg_d = nc.dram_tensor("msg", [NPC * K, D], F32, kind="ExternalInput").ap()
    typesf_d = nc.dram_tensor("typesf", [N_ST, 128, 16], F32, kind="ExternalInput").ap()
    cnt_d = nc.dram_tensor("cnt", [N_ST, 128, 128], F32, kind="ExternalInput").ap()
    w1_d = nc.dram_tensor("w1", [128, 64], F32, kind="ExternalInput").ap()
    w2_d = nc.dram_tensor("w2", [128, 64], F32, kind="ExternalInput").ap()
    bias_d = nc.dram_tensor("bias", [1, 128], F32, kind="ExternalInput").ap()
    iota_d = nc.dram_tensor("iota", [128, 128], F32, kind="ExternalInput").ap()
    bdm_d = nc.dram_tensor("bdmask", [128, 256], F32, kind="ExternalInput").ap()
    ones_d = nc.dram_tensor("ones", [1, 128], F32, kind="ExternalInput").ap()
    out_d = nc.dram_tensor("out", [NPC, 128], F32, kind="ExternalOutput").ap()

    # host pre-permutes msg to (st, j, k, g, d) order, so each supertile's
    # [128, 2048] tile is one fully contiguous 1 MB DMA
    msg_v = msg_d.rearrange("(st r) d -> st r d", r=2048)
    out_v = out_d.rearrange("(st p) j -> st p j", p=128)

    with tile.TileContext(nc) as tc:
        with ExitStack() as ctx:
            cpool = ctx.enter_context(tc.tile_pool(name="consts", bufs=1))
            mpool = ctx.enter_context(tc.tile_pool(name="msgp", bufs=6))
            spool = ctx.enter_context(tc.tile_pool(name="small", bufs=4))
            wpool = ctx.enter_context(tc.tile_pool(name="work", bufs=3))
            opool = ctx.enter_context(tc.tile_pool(name="outp", bufs=4))
            ppool = ctx.enter_context(
                tc.tile_pool(name="psred", bufs=3, space=bass.MemorySpace.PSUM)
            )
            p2pool = ctx.enter_context(
                tc.tile_pool(name="ps2", bufs=3, space=bass.MemorySpace.PSUM)
            )

            w1_sb = cpool.tile([128, 64], F32)
            nc.sync.dma_start(w1_sb[:], w1_d)
            w2_sb = cpool.tile([128, 64], F32)
            nc.sync.dma_start(w2_sb[:], w2_d)
            bias_sb = cpool.tile([1, 128], F32)
            nc.sync.dma_start(bias_sb[:], bias_d)
            iota_sb = cpool.tile([128, 128], F32)
            nc.sync.dma_start(iota_sb[:], iota_d)
            bdm_sb = cpool.tile([128, 256], F32)
            nc.sync.dma_start(bdm_sb[:], bdm_d)
            ones_sb = cpool.tile([1, 128], F32)
            nc.sync.dma_start(ones_sb[:], ones_d)

            for st in range(N_ST):
                msg_sb = mpool.tile([128, GROUPS * D], F32, tag="msg")
                nc.sync.dma_start(msg_sb[:], msg_v[st])
                tys = spool.tile([128, 16], F32, tag="tys")
                nc.scalar.dma_start(tys[:], typesf_d[st])
                cnt = spool.tile([128, 128], F32, tag="cnt")
                nc.scalar.dma_start(cnt[:], cnt_d[st])

                # one-hot gather of counts: gcnt = sum_e (type==e)*cnt
                mask = wpool.tile([128, 128], F32, tag="mask")
                nc.vector.tensor_tensor(
                    mask[:].rearrange("p (g e) -> p g e", e=E),
                    tys[:].unsqueeze(2).broadcast_to([128, GROUPS, E]),
                    iota_sb[:].rearrange("p (g e) -> p g e", e=E),
                    mybir.AluOpType.is_equal,
                )
                prodc = wpool.tile([128, 128], F32, tag="prodc")
                nc.vector.tensor_tensor(prodc[:], mask[:], cnt[:], mybir.AluOpType.mult)
                gcnt = wpool.tile([128, GROUPS], F32, tag="gcnt")
                nc.vector.tensor_reduce(
                    gcnt[:],
                    prodc[:].rearrange("p (g e) -> p g e", e=E),
                    mybir.AxisListType.X,
                    mybir.AluOpType.add,
                )
                tcnt = wpool.tile([128, GROUPS], F32, tag="tcnt")
                nc.vector.tensor_reduce(
                    tcnt[:],
                    cnt[:].rearrange("p (g e) -> p g e", e=E),
                    mybir.AxisListType.X,
                    mybir.AluOpType.add,
                )
                rq = wpool.tile([128, GROUPS * 2], F32, tag="rq")
                rq_v = rq[:].rearrange("p (g t) -> p g t", t=2)
                nc.vector.reciprocal(rq_v[:, :, 0], gcnt[:])
                nc.vector.reciprocal(rq_v[:, :, 1], tcnt[:])

                # block-diagonal rhs: ball[p,(g,t,r)] = bdmask[p,(t,r)] * rq[p,(g,t)]
                ball = wpool.tile([128, GROUPS * 16], F32, tag="ball")
                nc.vector.tensor_tensor(
                    ball[:].rearrange("p (g t r) -> p g t r", t=2, r=GNODES),
                    bdm_sb[:].rearrange("p (g t r) -> p g t r", t=2, r=GNODES),
                    rq_v.unsqueeze(3).broadcast_to([128, GROUPS, 2, GNODES]),
                    mybir.AluOpType.mult,
                )

                # stage 1: per-group K-reduction on PE (msg stationary)
                psum_red = ppool.tile([128, GROUPS * 16], F32, tag="pr")
                for g in range(GROUPS):
                    nc.tensor.matmul(
                        psum_red[:, g * 16 : (g + 1) * 16],
                        msg_sb[:, g * D : (g + 1) * D],
                        ball[:, g * 16 : (g + 1) * 16],
                        start=True,
                        stop=True,
                    )

                # evict + deinterleave: aT, qsT [128 d, 128 nodes]
                pr_v = psum_red[:].rearrange("p (g t r) -> p g t r", t=2, r=GNODES)
                aT = opool.tile([128, 128], F32, tag="aT")
                nc.scalar.activation(
                    aT[:].rearrange("p (g r) -> p g r", r=GNODES),
                    pr_v[:, :, 0, :],
                    mybir.ActivationFunctionType.Copy,
                )
                qsT = opool.tile([128, 128], F32, tag="qsT")
                nc.scalar.activation(
                    qsT[:].rearrange("p (g r) -> p g r", r=GNODES),
                    pr_v[:, :, 1, :],
                    mybir.ActivationFunctionType.Copy,
                )

                # stage 2: rank-1 bias + two linears accumulated in psum2
                psum2 = p2pool.tile([128, 128], F32, tag="p2")
                nc.tensor.matmul(
                    psum2[:, :], ones_sb[:, :], bias_sb[:, :], start=True, stop=False
                )
                nc.tensor.matmul(psum2[:, 0:64], aT[:], w1_sb[:], start=False, stop=False)
                nc.tensor.matmul(
                    psum2[:, 64:128], qsT[:], w2_sb[:], start=False, stop=True
                )

                out_sb = opool.tile([128, 128], F32, tag="osb")
                nc.scalar.activation(
                    out_sb[:], psum2[:], mybir.ActivationFunctionType.Copy
                )
                nc.scalar.dma_start(out_v[st], out_sb[:])

    nc.compile()
    return nc


def _get_program():
    global _PROG
    if _PROG is None:
        _PROG = _build_program()
    return _PROG


def _host_consts(W1, b1, W2, b2):
    iota = np.tile(np.arange(E, dtype=np.float32), (128, GROUPS))
    p = np.arange(128)[:, None]
    r = np.arange(GNODES)[None, :]
    bd16 = (p // 16 == r).astype(np.float32)
    bdmask = np.tile(np.concatenate([bd16, bd16], axis=1), (1, GROUPS))
    bias = np.concatenate([b1, b2]).astype(np.float32)[None, :]
    return {
        "w1": np.ascontiguousarray(W1, dtype=np.float32),
        "w2": np.ascontiguousarray(W2, dtype=np.float32),
        "bias": bias,
        "iota": iota,
        "bdmask": bdmask,
        "ones": np.ones((1, 128), np.float32),
    }


def _host_prep_core(msg_c, types_c, count0_c):
    # typesf[st, 16j+k, g] = types[st*128+8g+j, k]
    et = types_c.reshape(N_ST, GROUPS, GNODES, K).astype(np.float32)
    tf = np.transpose(et, (0, 2, 3, 1))  # [st,j,k,g]
    typesf = np.ascontiguousarray(tf).reshape(N_ST, 128, GROUPS)

    # cnt[st, 16j+k, 8g+e] = count0[st*128+8g+j, e]
    c0 = count0_c.reshape(N_ST, GROUPS, GNODES, E).astype(np.float32)
    cf = np.transpose(c0, (0, 2, 1, 3))  # [st,j,g,e]
    cf = np.broadcast_to(cf[:, :, None, :, :], (N_ST, GNODES, K, GROUPS, E))
    cnt = np.ascontiguousarray(cf).reshape(N_ST, 128, 128)

    # permute msg rows from (st, g, j, k) to (st, j, k, g) so each
    # supertile's [128=(j,k), (g,d)] tile is contiguous in DRAM
    mv = np.asarray(msg_c, dtype=np.float32).reshape(N_ST, GROUPS, GNODES, K, D)
    msg_perm = np.ascontiguousarray(np.transpose(mv, (0, 2, 3, 1, 4)))

    return {
        "msg": msg_perm.reshape(NPC * K, D),
        "typesf": typesf,
        "cnt": cnt,
    }


def _make_in_maps(msg, e_type, e_count, W1, b1, W2, b2):
    msg = np.asarray(msg, dtype=np.float32)
    e_type = np.asarray(e_type)
    count0 = np.ascontiguousarray(np.asarray(e_count, dtype=np.float32)[:, 0, :])

    consts = _host_consts(
        np.asarray(W1), np.asarray(b1, dtype=np.float32),
        np.asarray(W2), np.asarray(b2, dtype=np.float32),
    )

    in_maps = []
    for c in range(N_CORES):
        lo, hi = c * NPC, (c + 1) * NPC
        if hi <= N:
            m_c = msg[lo:hi]
            t_c = e_type[lo:hi]
            c_c = count0[lo:hi]
        else:
            m_c = np.zeros((NPC, K, D), np.float32)
            m_c[: N - lo] = msg[lo:N]
            t_c = np.zeros((NPC, K), e_type.dtype)
            t_c[: N - lo] = e_type[lo:N]
            c_c = np.ones((NPC, E), np.float32)
            c_c[: N - lo] = count0[lo:N]
        im = _host_prep_core(m_c, t_c, c_c)
        im.update(consts)
        in_maps.append(im)
    return in_maps


_RUNNER = None


def _get_runner():
    """Build (once) a jitted shard_map callable over the 8 cores.

    Mirrors concourse.bass2jax.run_bass_via_pjrt's multi-core branch but
    caches the jitted function so repeat calls don't re-trace/re-jit.
    Returns (fn, in_names, out_names, out_avals, n_params).
    """
    global _RUNNER
    if _RUNNER is not None:
        return _RUNNER

    import jax
    from jax.sharding import Mesh, PartitionSpec
    from jax.experimental.shard_map import shard_map
    from concourse import bass2jax, mybir

    bass2jax.install_neuronx_cc_hook()
    nc = _get_program()
    partition_name = nc.partition_id_tensor.name if nc.partition_id_tensor else None

    in_names, out_names, out_avals, zero_outs = [], [], [], []
    for alloc in nc.m.functions[0].allocations:
        if not isinstance(alloc, mybir.MemoryLocationSet):
            continue
        name = alloc.memorylocations[0].name
        if alloc.kind == "ExternalInput":
            if name != partition_name:
                in_names.append(name)
        elif alloc.kind == "ExternalOutput":
            shape = tuple(alloc.tensor_shape)
            dtype = mybir.dt.np(alloc.dtype)
            out_names.append(name)
            out_avals.append(jax.core.ShapedArray(shape, dtype))
            zero_outs.append(np.zeros(shape, dtype))
    n_params = len(in_names)
    n_outs = len(out_avals)
    in_names = in_names + out_names
    if partition_name is not None:
        in_names.append(partition_name)
    donate = tuple(range(n_params, n_params + n_outs))

    def _body(*args):
        operands = list(args)
        if partition_name is not None:
            operands.append(bass2jax.partition_id_tensor())
        outs = bass2jax._bass_exec_p.bind(
            *operands,
            out_avals=tuple(out_avals),
            in_names=tuple(in_names),
            out_names=tuple(out_names),
            lowering_input_output_aliases=(),
            sim_require_finite=True,
            sim_require_nnan=True,
            nc=nc,
        )
        return tuple(outs)

    devices = jax.devices()[:N_CORES]
    mesh = Mesh(np.asarray(devices), ("core",))
    in_specs = (PartitionSpec("core"),) * (n_params + n_outs)
    out_specs = (PartitionSpec("core"),) * n_outs
    fn = jax.jit(
        shard_map(
            _body, mesh=mesh, in_specs=in_specs, out_specs=out_specs, check_rep=False
        ),
        donate_argnums=donate,
        keep_unused=True,
    )
    _RUNNER = (fn, in_names, out_names, out_avals, n_params, zero_outs, mesh)
    return _RUNNER


def _concat_inputs(in_maps, in_names, n_params):
    return [
        np.concatenate([np.asarray(in_maps[c][nm]) for c in range(N_CORES)], axis=0)
        for nm in in_names[:n_params]
    ]


def kernel(msg, e_type, e_count, W1, b1, W2, b2):
    fn, in_names, out_names, out_avals, n_params, zero_outs, _mesh = _get_runner()
    in_maps = _make_in_maps(msg, e_type, e_count, W1, b1, W2, b2)
    concat_in = _concat_inputs(in_maps, in_names, n_params)
    def _run_once():
        concat_zeros = [
            np.zeros((N_CORES * z.shape[0], *z.shape[1:]), z.dtype) for z in zero_outs
        ]
        arrs = fn(*concat_in, *concat_zeros)
        return [np.asarray(a) for a in arrs]

    try:
        out_arrs = _run_once()
    except Exception:
        # transient relay/device hiccups (e.g. NRT_EXEC_UNIT_UNRECOVERABLE)
        # have been observed to clear on a sequential retry
        import time as _time

        _time.sleep(5.0)
        out_arrs = _run_once()
    oi = out_names.index("out")
    out = np.asarray(out_arrs[oi])  # [N_CORES*NPC, 128]
    return np.ascontiguousarray(out[:N])



# revision 2
# speedup vs baseline: 1.6532x; 1.6532x over previous
"""NormalizeAggregator TRN2 Bass kernel (8-core SPMD, node-sharded), v2.

kernel(**inputs) -> np.ndarray [100000, 128] float32

Math per node n (K=16 mailbox slots, D=128, E=8 edge types):
  count0[n,:]  = e_count[n,0,:]
  T[n]         = sum_e count0[n,e]
  r[n,k]       = 1 / count0[n, e_type[n,k]]
  a[n,:]       = sum_k r[n,k] * msg[n,k,:]
  s[n,:]       = (sum_k msg[n,k,:]) / T[n]
  out[n,:]     = [a @ W1 + b1 , s @ W2 + b2]

v2 changes over the baseline:
  * bf16 everywhere on device (msg, weights, biases); f32 PSUM accumulate.
    Halves the HBM traffic for msg — the memory-regime bottleneck.
  * r[n,k] and q[n]=1/T[n] are gathered/inverted on the host (cheap [N,K]
    numpy) and shipped packed, so the on-device one-hot gather (6 vector
    ops / supertile) disappears. All per-ST rq weights live resident in
    SBUF from one upfront DMA.
  * all inputs packed into ONE 1-D bf16 blob per core (fewer per-call
    buffer bindings on the PJRT/axon dispatch path).

Device layout (per core): supertile = 128 nodes = 16 groups x 8 nodes.
Per group the msg block [(8j x 16k) = 128, 128d] is the matmul lhsT;
rhs is a [128, 16] block-diagonal weight (8 cols of r, 8 cols of q) so
psum holds a^T / (q*s)^T directly. Stage 2 accumulates rank-1 bias +
aT.T @ W1 | qsT.T @ W2 into one [128 nodes, 128] psum tile.
"""

import sys
from contextlib import ExitStack

import numpy as np

if "/opt/trn_rl_repo" not in sys.path:
    sys.path.insert(0, "/opt/trn_rl_repo")

import ml_dtypes

BF16 = ml_dtypes.bfloat16

N = 100000
K = 16
D = 128
E = 8
N_CORES = 8
NODES_PER_ST = 128
GROUPS = 16
GNODES = 8
N_ST = 98  # supertiles per core
NPC = N_ST * NODES_PER_ST  # 12544 nodes per core
N_PAD = NPC * N_CORES  # 100352

# blob element offsets (bf16 elements)
_MSG_LEN = N_ST * 128 * (GROUPS * D)  # 25690112
_RQ_LEN = 128 * (N_ST * GROUPS * 2)  # 401408
_BDM_LEN = 128 * 256
_W_LEN = 128 * 64
_B_LEN = 128
OFF_MSG = 0
OFF_RQ = OFF_MSG + _MSG_LEN
OFF_BDM = OFF_RQ + _RQ_LEN
OFF_W1 = OFF_BDM + _BDM_LEN
OFF_W2 = OFF_W1 + _W_LEN
OFF_BIAS = OFF_W2 + _W_LEN
OFF_ONES = OFF_BIAS + _B_LEN
BLOB_LEN = OFF_ONES + _B_LEN

_PROG = None


def _build_program():
    from concourse import bacc, bass, mybir, tile

    F32 = mybir.dt.float32
    BF = mybir.dt.bfloat16
    nc = bacc.Bacc("TRN2", target_bir_lowering=False, debug=False, num_devices=N_CORES)

    blob_d = nc.dram_tensor("blob", [BLOB_LEN], BF, kind="ExternalInput").ap()
    out_d = nc.dram_tensor("out", [NPC, 128], F32, kind="ExternalOutput").ap()

    msg_v = blob_d[OFF_MSG:OFF_RQ].rearrange(
        "(st p c) -> st p c", p=128, c=GROUPS * D
    )
    rq_dv = blob_d[OFF_RQ:OFF_BDM].rearrange("(p c) -> p c", c=N_ST * GROUPS * 2)
    bdm_dv = blob_d[OFF_BDM:OFF_W1].rearrange("(p c) -> p c", c=256)
    w1_dv = blob_d[OFF_W1:OFF_W2].rearrange("(p c) -> p c", c=64)
    w2_dv = blob_d[OFF_W2:OFF_BIAS].rearrange("(p c) -> p c", c=64)
    bias_dv = blob_d[OFF_BIAS:OFF_ONES].rearrange("(p c) -> p c", c=128)
    ones_dv = blob_d[OFF_ONES:BLOB_LEN].rearrange("(p c) -> p c", c=128)
    out_v = out_d.rearrange("(st p) j -> st p j", p=128)

    with tile.TileContext(nc) as tc:
        with ExitStack() as ctx:
            cpool = ctx.enter_context(tc.tile_pool(name="consts", bufs=1))
            mpool = ctx.enter_context(tc.tile_pool(name="msgp", bufs=6))
            wpool = ctx.enter_context(tc.tile_pool(name="work", bufs=4))
            opool = ctx.enter_context(tc.tile_pool(name="outp", bufs=4))
            ppool = ctx.enter_context(
                tc.tile_pool(name="psred", bufs=3, space=bass.MemorySpace.PSUM)
            )
            p2pool = ctx.enter_context(
                tc.tile_pool(name="ps2", bufs=3, space=bass.MemorySpace.PSUM)
            )

            rq_sb = cpool.tile([128, N_ST * GROUPS * 2], BF)
            nc.sync.dma_start(rq_sb[:], rq_dv)
            bdm_sb = cpool.tile([128, 256], BF)
            nc.sync.dma_start(bdm_sb[:], bdm_dv)
            w1_sb = cpool.tile([128, 64], BF)
            nc.sync.dma_start(w1_sb[:], w1_dv)
            w2_sb = cpool.tile([128, 64], BF)
            nc.sync.dma_start(w2_sb[:], w2_dv)
            bias_sb = cpool.tile([1, 128], BF)
            nc.sync.dma_start(bias_sb[:], bias_dv[0:1])
            ones_sb = cpool.tile([1, 128], BF)
            nc.sync.dma_start(ones_sb[:], ones_dv[0:1])

            for st in range(N_ST):
                msg_sb = mpool.tile([128, GROUPS * D], BF, tag="msg")
                nc.sync.dma_start(msg_sb[:], msg_v[st])

                # block-diagonal rhs from resident host-computed rq:
                # ball[p,(g,t,r)] = bdmask[p,(t,r)] * rq[p,(st,g,t)]
                ball = wpool.tile([128, GROUPS * 16], BF, tag="ball")
                nc.vector.tensor_tensor(
                    ball[:].rearrange("p (g t r) -> p g t r", t=2, r=GNODES),
                    bdm_sb[:].rearrange("p (g t r) -> p g t r", t=2, r=GNODES),
                    rq_sb[:, st * 32 : (st + 1) * 32]
                    .rearrange("p (g t) -> p g t", t=2)
                    .unsqueeze(3)
                    .broadcast_to([128, GROUPS, 2, GNODES]),
                    mybir.AluOpType.mult,
                )

                # stage 1: per-group K-reduction on PE (msg stationary)
                psum_red = ppool.tile([128, GROUPS * 16], F32, tag="pr")
                for g in range(GROUPS):
                    nc.tensor.matmul(
                        psum_red[:, g * 16 : (g + 1) * 16],
                        msg_sb[:, g * D : (g + 1) * D],
                        ball[:, g * 16 : (g + 1) * 16],
                        start=True,
                        stop=True,
                    )

                # evict + deinterleave: aT, qsT [128 d, 128 nodes] (bf16)
                pr_v = psum_red[:].rearrange("p (g t r) -> p g t r", t=2, r=GNODES)
                aT = opool.tile([128, 128], BF, tag="aT")
                nc.scalar.activation(
                    aT[:].rearrange("p (g r) -> p g r", r=GNODES),
                    pr_v[:, :, 0, :],
                    mybir.ActivationFunctionType.Copy,
                )
                qsT = opool.tile([128, 128], BF, tag="qsT")
                nc.scalar.activation(
                    qsT[:].rearrange("p (g r) -> p g r", r=GNODES),
                    pr_v[:, :, 1, :],
                    mybir.ActivationFunctionType.Copy,
                )

                # stage 2: rank-1 bias + two linears accumulated in psum2
                psum2 = p2pool.tile([128, 128], F32, tag="p2")
                nc.tensor.matmul(
                    psum2[:, :], ones_sb[:, :], bias_sb[:, :], start=True, stop=False
                )
                nc.tensor.matmul(psum2[:, 0:64], aT[:], w1_sb[:], start=False, stop=False)
                nc.tensor.matmul(
                    psum2[:, 64:128], qsT[:], w2_sb[:], start=False, stop=True
                )

                out_sb = opool.tile([128, 128], F32, tag="osb")
                nc.scalar.activation(
                    out_sb[:], psum2[:], mybir.ActivationFunctionType.Copy
                )
                nc.scalar.dma_start(out_v[st], out_sb[:])

    nc.compile()
    return nc


def _get_program():
    global _PROG
    if _PROG is None:
        _PROG = _build_program()
    return _PROG


def _host_consts(W1, b1, W2, b2):
    p = np.arange(128)[:, None]
    r = np.arange(GNODES)[None, :]
    bd16 = (p // 16 == r).astype(np.float32)
    bdmask = np.tile(np.concatenate([bd16, bd16], axis=1), (1, GROUPS))
    bias = np.concatenate(
        [np.asarray(b1, np.float32), np.asarray(b2, np.float32)]
    )[None, :]
    return {
        "bdm": bdmask.astype(BF16),
        "w1": np.ascontiguousarray(np.asarray(W1, np.float32)).astype(BF16),
        "w2": np.ascontiguousarray(np.asarray(W2, np.float32)).astype(BF16),
        "bias": np.broadcast_to(bias.astype(BF16), (128, 128)).copy(),
        "ones": np.ones((128, 128), BF16),
    }


def _host_prep_core(msg_c, rw_c, qw_c, consts):
    """Build the packed bf16 blob for one core.

    msg rows permuted from (st, g, j, k) to (st, j, k, g) so each
    supertile's [128=(j,k), (g,d)] tile is contiguous in DRAM.
    rq packed as [p=(j,k), (st, g, t)]; t=0 -> r-weight, t=1 -> q.
    """
    mv = msg_c.reshape(N_ST, GROUPS, GNODES, K, D)
    msg_perm = np.ascontiguousarray(np.transpose(mv, (0, 2, 3, 1, 4)))

    rw = rw_c.reshape(N_ST, GROUPS, GNODES, K)  # (st, g, j, k)
    qw = qw_c.reshape(N_ST, GROUPS, GNODES)  # (st, g, j)
    qb = np.broadcast_to(qw[:, :, :, None], (N_ST, GROUPS, GNODES, K))
    rq = np.stack([rw, qb], axis=-1)  # (st, g, j, k, t)
    # -> (j, k, st, g, t)
    rq = np.ascontiguousarray(np.transpose(rq, (2, 3, 0, 1, 4)))

    blob = np.empty(BLOB_LEN, BF16)
    blob[OFF_MSG:OFF_RQ] = msg_perm.reshape(-1)
    blob[OFF_RQ:OFF_BDM] = rq.reshape(-1)
    blob[OFF_BDM:OFF_W1] = consts["bdm"].reshape(-1)
    blob[OFF_W1:OFF_W2] = consts["w1"].reshape(-1)
    blob[OFF_W2:OFF_BIAS] = consts["w2"].reshape(-1)
    blob[OFF_BIAS:OFF_ONES] = consts["bias"][0].reshape(-1)
    blob[OFF_ONES:BLOB_LEN] = consts["ones"][0].reshape(-1)
    return {"blob": blob}


def _make_in_maps(msg, e_type, e_count, W1, b1, W2, b2):
    msg = np.asarray(msg, dtype=np.float32)
    e_type = np.asarray(e_type)
    count0 = np.ascontiguousarray(np.asarray(e_count, dtype=np.float32)[:, 0, :])

    # host-side gather: per-edge weight r and per-node total-inverse q
    rw = (1.0 / np.take_along_axis(count0, e_type.astype(np.int64), axis=-1)).astype(
        BF16
    )  # [N, K]
    qw = (1.0 / count0.sum(axis=-1)).astype(BF16)  # [N]

    msg_bf = msg.astype(BF16)
    consts = _host_consts(W1, b1, W2, b2)

    in_maps = []
    for c in range(N_CORES):
        lo, hi = c * NPC, (c + 1) * NPC
        if hi <= N:
            m_c = msg_bf[lo:hi]
            r_c = rw[lo:hi]
            q_c = qw[lo:hi]
        else:
            m_c = np.zeros((NPC, K, D), BF16)
            m_c[: N - lo] = msg_bf[lo:N]
            r_c = np.ones((NPC, K), BF16)
            r_c[: N - lo] = rw[lo:N]
            q_c = np.ones((NPC,), BF16)
            q_c[: N - lo] = qw[lo:N]
        in_maps.append(_host_prep_core(m_c, r_c, q_c, consts))
    return in_maps


_RUNNER = None


def _get_runner():
    """Build (once) a jitted shard_map callable over the 8 cores."""
    global _RUNNER
    if _RUNNER is not None:
        return _RUNNER

    import jax
    from jax.sharding import Mesh, PartitionSpec
    from jax.experimental.shard_map import shard_map
    from concourse import bass2jax, mybir

    bass2jax.install_neuronx_cc_hook()
    nc = _get_program()
    partition_name = nc.partition_id_tensor.name if nc.partition_id_tensor else None

    in_names, out_names, out_avals, zero_outs = [], [], [], []
    for alloc in nc.m.functions[0].allocations:
        if not isinstance(alloc, mybir.MemoryLocationSet):
            continue
        name = alloc.memorylocations[0].name
        if alloc.kind == "ExternalInput":
            if name != partition_name:
                in_names.append(name)
        elif alloc.kind == "ExternalOutput":
            shape = tuple(alloc.tensor_shape)
            dtype = mybir.dt.np(alloc.dtype)
            out_names.append(name)
            out_avals.append(jax.core.ShapedArray(shape, dtype))
            zero_outs.append(np.zeros(shape, dtype))
    n_params = len(in_names)
    n_outs = len(out_avals)
    in_names = in_names + out_names
    if partition_name is not None:
        in_names.append(partition_name)
    donate = tuple(range(n_params, n_params + n_outs))

    def _body(*args):
        operands = list(args)
        if partition_name is not None:
            operands.append(bass2jax.partition_id_tensor())
        outs = bass2jax._bass_exec_p.bind(
            *operands,
            out_avals=tuple(out_avals),
            in_names=tuple(in_names),
            out_names=tuple(out_names),
            lowering_input_output_aliases=(),
            sim_require_finite=True,
            sim_require_nnan=True,
            nc=nc,
        )
        return tuple(outs)

    devices = jax.devices()[:N_CORES]
    mesh = Mesh(np.asarray(devices), ("core",))
    in_specs = (PartitionSpec("core"),) * (n_params + n_outs)
    out_specs = (PartitionSpec("core"),) * n_outs
    fn = jax.jit(
        shard_map(
            _body, mesh=mesh, in_specs=in_specs, out_specs=out_specs, check_rep=False
        ),
        donate_argnums=donate,
        keep_unused=True,
    )
    _RUNNER = (fn, in_names, out_names, out_avals, n_params, zero_outs, mesh)
    return _RUNNER


def _concat_inputs(in_maps, in_names, n_params):
    return [
        np.concatenate([np.asarray(in_maps[c][nm]) for c in range(N_CORES)], axis=0)
        for nm in in_names[:n_params]
    ]


def kernel(msg, e_type, e_count, W1, b1, W2, b2):
    fn, in_names, out_names, out_avals, n_params, zero_outs, _mesh = _get_runner()
    in_maps = _make_in_maps(msg, e_type, e_count, W1, b1, W2, b2)
    concat_in = _concat_inputs(in_maps, in_names, n_params)

    def _run_once():
        concat_zeros = [
            np.zeros((N_CORES * z.shape[0], *z.shape[1:]), z.dtype) for z in zero_outs
        ]
        arrs = fn(*concat_in, *concat_zeros)
        return [np.asarray(a) for a in arrs]

    try:
        out_arrs = _run_once()
    except Exception:
        # transient relay/device hiccups have been observed to clear on retry
        import time as _time

        _time.sleep(5.0)
        out_arrs = _run_once()
    oi = out_names.index("out")
    out = np.asarray(out_arrs[oi])  # [N_CORES*NPC, 128]
    return np.ascontiguousarray(out[:N])
